# revision 12
# baseline (speedup 1.0000x reference)
"""CliffordSpectralConv2d on 8 trn2 NeuronCores (axon-tunneled).

Math: per sample b and "dual pair" (d1 = x0 + i*x3, d2 = x1 + i*x2):
    Y_d   = A @ X_d @ A^T          (crop-DFT, A = F256[rows 0:32 + 224:256])
    OD1   = sum_c W1*Y1 + W2*conj(Y2)   (positionwise over the 64x64 modes)
    OD2   = sum_c W1*Y2 + W2*conj(Y1)   (W1 = w0 + i*w3, W2 = w1 + i*w2)
    out_d = (1/65536) A^H @ OD_d @ conj(A)
with out components (re(o1), re(o2), im(o2), im(o1)).

Sharding (8 cores, one NEFF, SPMD):
  core k = (b = k%4, half = k//4)
  phase F: forward crop-DFT for x[b, 16*half:16*half+16] (32 complex ch)
  AllToAll #1 (1 MB/rank): reshard Y by mode-row slices
  phase M: per mode-row, y'_g = pg[g]^T y (4 signed-permutation matmuls,
           with per-channel input-dequant scales folded into pg), then
           positionwise contraction against the compact int8 weight table
           Tall[(s,c), (pos,o)] -- built host-side with NO 4x blade
           expansion (16.8 MB total over the wire instead of 134 MB).
  AllToAll #2 (1 MB/rank): reshard OD by (b, out-channel-half)
  phase I: inverse DFT for 16 output channels, interleave components,
           quantize each [128, W*4] row-tile to int8 with a per-row
           absmax scale (scales returned separately).

Wire-format (the axon tunnel runs at ~35 MB/s, so bytes == time):
  H2D per call: xs int8 (33.5 MB, per-(channel,pair) scales folded into
  pg), wq int8 (16.8 MB), pg fp32 (2 MB).  D2H: oq int8 (33.5 MB) +
  oscale fp32 (0.13 MB).  DFT matrices ride inside the NEFF as Const
  tensors; the jitted executable is cached across calls.
"""

import threading

import numpy as np

import jax

import concourse.bass as bass  # noqa: F401  (kept for parity with docs)
import concourse.mybir as mybir
import concourse.tile as tile
from concourse import bacc
from concourse import bass2jax as _b2j

NCORES = 8
B, CIN, COUT, H, W = 4, 32, 32, 256, 256
M = 32            # modes per corner
M2 = 64           # 2*M
CH = 16           # channels per core (forward)
OH = 16           # out channels per core (inverse)
ROWS = 8          # mode rows per core (mix)
POS = ROWS * M2   # positions per core (512)

FP32 = mybir.dt.float32
BF16 = mybir.dt.bfloat16
INT8 = mybir.dt.int8

MIX_DT = BF16     # mode-mix matmul dtype

# baseline grid: (matrix index in [W1r, W1i, W2r, W2i] order, sign),
# rows = input blade b in [Y1r, Y1i, Y2r, Y2i], cols = output blade g in
# [OD1r, OD1i, OD2r, OD2i].  W1r=w[0], W1i=w[3], W2r=w[1], W2i=w[2].
GRID = [
    [(0, 1.0), (1, 1.0), (2, 1.0), (3, 1.0)],
    [(1, -1.0), (0, 1.0), (3, 1.0), (2, -1.0)],
    [(2, 1.0), (3, 1.0), (0, 1.0), (1, 1.0)],
    [(3, 1.0), (2, -1.0), (1, -1.0), (0, 1.0)],
]

_prep_cache = {}


def _dft_mats():
    k = np.arange(H)
    sel = np.concatenate([np.arange(M), np.arange(H - M, H)])
    F = np.exp(-2j * np.pi * np.outer(k, k) / H)
    A = F[sel, :]
    return A.real.astype(np.float32).copy(), A.imag.astype(np.float32).copy()


def _host_consts():
    Ar, Ai = _dft_mats()  # (64, 256)
    # rx[ck, 0] = [Ar_chunk^T | Ai_chunk^T]; rx[ck, 1] = [-Ai_chunk^T | Ar_chunk^T]
    rx = np.zeros((2, 2, 128, 128), np.float32)
    for ck in range(2):
        ArT = Ar[:, ck * 128:(ck + 1) * 128].T  # (128, 64)
        AiT = Ai[:, ck * 128:(ck + 1) * 128].T
        rx[ck, 0, :, :64], rx[ck, 0, :, 64:] = ArT, AiT
        rx[ck, 1, :, :64], rx[ck, 1, :, 64:] = -AiT, ArT
    # ia[hb, 0] = [Ar_chunk; Ai_chunk] rows; ia[hb, 1] = [-Ai_chunk; Ar_chunk]
    ia = np.zeros((2, 2, 128, 128), np.float32)
    for hb in range(2):
        Arc = Ar[:, hb * 128:(hb + 1) * 128]  # (64, 128)
        Aic = Ai[:, hb * 128:(hb + 1) * 128]
        ia[hb, 0, :64], ia[hb, 0, 64:] = Arc, Aic
        ia[hb, 1, :64], ia[hb, 1, 64:] = -Aic, Arc
    # ib[0] = [Ar; Ai]/65536 ; ib[1] = [-Ai; Ar]/65536   (128, 256)
    s = 1.0 / float(H * W)
    ib = np.zeros((2, 128, 256), np.float32)
    ib[0, :64], ib[0, 64:] = Ar * s, Ai * s
    ib[1, :64], ib[1, 64:] = -Ai * s, Ar * s
    ident = np.eye(128, dtype=np.float32)
    return rx, ia, ib, ident


def _quantize_inputs(x, weights):
    """int8 wire format + the dequant data folded where the device needs it.

    Returns (xs_global, wq_global, pg_global, s_w) where the *_global arrays
    are the concatenated per-core shards for shard_map axis 0.
    """
    x = np.asarray(x, np.float32)
    w = np.asarray(weights, np.float32)

    # per-(channel, dual-pair) scales; pair d1 = comps (0,3), d2 = (1,2)
    ax = np.abs(x)
    m03 = np.maximum(ax[..., 0], ax[..., 3]).max(axis=(0, 2, 3))  # (C,)
    m12 = np.maximum(ax[..., 1], ax[..., 2]).max(axis=(0, 2, 3))
    s_cd = np.stack([m03, m12], axis=1) / 127.0                   # (C, 2)
    s_cd = np.maximum(s_cd, 1e-30)
    sc = s_cd[:, [0, 1, 1, 0]]                                    # per comp k
    xq = np.clip(np.round(x / sc[None, :, None, None, :]), -127, 127)
    xq = xq.astype(np.int8)
    # core k = (b = k%4, half = k//4) -> rows [k*CH:(k+1)*CH]
    xs_global = np.empty((NCORES * CH, H, W, 4), np.int8)
    for k in range(NCORES):
        b, half = k % 4, k // 4
        xs_global[k * CH:(k + 1) * CH] = xq[b, half * CH:(half + 1) * CH]

    # compact weight table: tall[s*32+c, pos*32+o] = w_sel[s][o, c, m1, m2]
    w_sel = w[[0, 3, 1, 2]]                                       # (4, O, C, 64, 64)
    s_w = max(float(np.abs(w_sel).max()) / 127.0, 1e-30)
    wq = np.clip(np.round(w_sel / s_w), -127, 127).astype(np.int8)
    t = np.ascontiguousarray(wq.transpose(0, 2, 3, 4, 1))         # (s, c, m1, m2, o)
    t = t.reshape(128, 64 * M2, 32)                               # rows (s,c), (pos, o)
    wq_global = np.empty((NCORES * 128, POS * 32), np.int8)
    for k in range(NCORES):
        wq_global[k * 128:(k + 1) * 128] = (
            t[:, POS * k:POS * (k + 1), :].reshape(128, POS * 32))

    # per-row dequant scales for the device-side signed-permutation const:
    # yb row i = (blade b)*32 + c carries scale s_cd[c, b//2]
    svec = np.empty((128, 1), np.float32)
    for i in range(128):
        svec[i, 0] = s_cd[i % 32, (i // 32) // 2]
    sv_global = np.tile(svec, (NCORES, 1, 1)).reshape(NCORES * 128, 1)

    return xs_global, wq_global, np.ascontiguousarray(sv_global), s_w


def _pg_pattern():
    """pg[g][i, k]: +-1 blade permutation, i = b*32+c, k = s*32+c."""
    pg = np.zeros((4, 128, 128), np.float32)
    for bb in range(4):
        for g in range(4):
            si, sign = GRID[bb][g]
            for c in range(32):
                pg[g, bb * 32 + c, si * 32 + c] = sign
    return pg


def _emit(nc, dbg=False):
    """Emit the SPMD program (same for every core; data differs)."""
    xs = nc.dram_tensor("xs", [CH, H, W, 4], INT8, kind="ExternalInput").ap()
    wq = nc.dram_tensor("wq", [128, POS * 32], INT8, kind="ExternalInput").ap()
    sv = nc.dram_tensor("sv", [128, 1], FP32, kind="ExternalInput").ap()
    pgt = nc.inline_tensor(_pg_pattern(), name="pgc").ap()
    oq = nc.dram_tensor("oq", [OH, H, W, 4], INT8, kind="ExternalOutput").ap()
    osc = nc.dram_tensor("osc", [OH, 2, 128], FP32, kind="ExternalOutput").ap()

    rxc, iac, ibc, identc = _host_consts()
    rx = nc.inline_tensor(rxc, name="rxc").ap()
    ia = nc.inline_tensor(iac, name="iac").ap()
    ib = nc.inline_tensor(ibc, name="ibc").ap()
    ident = nc.inline_tensor(identc, name="identc").ap()

    if dbg:
        ydbg = nc.dram_tensor("ydbg", [64, 2, 2, CH, M2], FP32, kind="ExternalOutput").ap()
        yhdbg = nc.dram_tensor("yhdbg", [ROWS, 128, 256], FP32, kind="ExternalOutput").ap()
        sdbg = nc.dram_tensor("sdbg", [4, 128, 512], FP32, kind="ExternalOutput").ap()
        bdbg = nc.dram_tensor("bdbg", [8, 4, OH, 8, M2], FP32, kind="ExternalOutput").ap()

    with tile.TileContext(nc) as tc:
        with (
            tc.tile_pool(name="consts", bufs=1) as cpool,
            tc.tile_pool(name="dram", bufs=1, space="DRAM") as dpool,
        ):
            # resident constants (partition dim must be first -> one tile each)
            rxs, ias = {}, {}
            for ck in range(2):
                for j in range(2):
                    t = cpool.tile([128, 128], FP32, name=f"rxs{ck}{j}")
                    nc.sync.dma_start(out=t[:], in_=rx[ck, j])
                    rxs[ck, j] = t
                    t2 = cpool.tile([128, 128], FP32, name=f"ias{ck}{j}")
                    nc.sync.dma_start(out=t2[:], in_=ia[ck, j])
                    ias[ck, j] = t2
            ibs = {}
            for j in range(2):
                t = cpool.tile([128, 256], FP32, name=f"ibs{j}")
                nc.sync.dma_start(out=t[:], in_=ib[j])
                ibs[j] = t
            ids = cpool.tile([128, 128], FP32, name="ids")
            nc.sync.dma_start(out=ids[:], in_=ident[:])
            svt = cpool.tile([128, 1], FP32, name="svt")
            nc.sync.dma_start(out=svt[:], in_=sv[:])
            pgs = {}
            for g in range(4):
                tb = cpool.tile([128, 128], FP32, name=f"pgb{g}")
                nc.sync.dma_start(out=tb[:], in_=pgt[g])
                t = cpool.tile([128, 128], FP32, name=f"pgs{g}")
                nc.vector.tensor_scalar(
                    out=t[:], in0=tb[:], scalar1=svt[:], scalar2=None,
                    op0=mybir.AluOpType.mult)
                pgs[g] = t
            # compact weight table -> resident bf16 Tall
            tall = cpool.tile([128, POS * 32], MIX_DT, name="tall")
            with tc.tile_pool(name="wload", bufs=1) as wl:
                ti = wl.tile([128, POS * 32], INT8, name="ti")
                nc.sync.dma_start(out=ti[:], in_=wq[:])
                nc.vector.tensor_copy(tall[:], ti[:])

            # collective buffers
            ybuf = dpool.tile([64, 2, 2, CH, M2], FP32, name="ybuf")
            arecv = dpool.tile([8, ROWS, 2, 2, CH, M2], FP32, name="arecv")
            bsend = dpool.tile([8, 4, OH, 4, 128], FP32, name="bsend")
            brecv = dpool.tile([8, 4, OH, 8, M2], FP32, name="brecv")

            # ---------------- phase F: forward crop-DFT ----------------
            with (
                tc.tile_pool(name="fsb", bufs=3) as fsb,
                tc.tile_pool(name="ftt", bufs=2) as ftt,
                tc.tile_pool(name="fps", bufs=2, space="PSUM") as fps,
            ):
                for c in range(CH):
                    xt0i = fsb.tile([128, W * 4], INT8, tag="xt0i")
                    nc.sync.dma_start(
                        out=xt0i[:], in_=xs[c, 0:128].rearrange("h w k -> h (w k)"))
                    xt0 = fsb.tile([128, W * 4], FP32, tag="xt0")
                    nc.vector.tensor_copy(xt0[:], xt0i[:])
                    xt1i = fsb.tile([128, W * 4], INT8, tag="xt1i")
                    nc.sync.dma_start(
                        out=xt1i[:], in_=xs[c, 128:256].rearrange("h w k -> h (w k)"))
                    xt1 = fsb.tile([128, W * 4], FP32, tag="xt1")
                    nc.vector.tensor_copy(xt1[:], xt1i[:])
                    xv = [xt0.rearrange("h (w k) -> h k w", k=4),
                          xt1.rearrange("h (w k) -> h k w", k=4)]
                    for d in range(2):
                        re_c, im_c = (0, 3) if d == 0 else (1, 2)
                        tts = []
                        for wb in range(2):
                            pt = fps.tile([128, 128], FP32, tag="pt")
                            for hk in range(2):
                                nc.tensor.matmul(
                                    pt[:],
                                    lhsT=xv[hk][:, re_c, wb * 128:(wb + 1) * 128],
                                    rhs=rxs[hk, 0][:],
                                    start=(hk == 0), stop=False)
                                nc.tensor.matmul(
                                    pt[:],
                                    lhsT=xv[hk][:, im_c, wb * 128:(wb + 1) * 128],
                                    rhs=rxs[hk, 1][:],
                                    start=False, stop=(hk == 1))
                            tt = ftt.tile([128, 128], FP32, tag=f"tt{wb}")
                            nc.vector.tensor_copy(tt[:], pt[:])
                            tts.append(tt)
                        py = fps.tile([64, 128], FP32, tag="py")
                        for wb in range(2):
                            nc.tensor.matmul(
                                py[:], lhsT=tts[wb][:, 0:64], rhs=rxs[wb, 0][:],
                                start=(wb == 0), stop=False)
                            nc.tensor.matmul(
                                py[:], lhsT=tts[wb][:, 64:128], rhs=rxs[wb, 1][:],
                                start=False, stop=(wb == 1))
                        sy = ftt.tile([64, 128], FP32, tag="sy")
                        nc.vector.tensor_copy(sy[:], py[:])
                        nc.sync.dma_start(
                            out=ybuf[:, d, :, c, :],
                            in_=sy.rearrange("m (r n) -> m r n", r=2))

            nc.gpsimd.collective_compute(
                "AllToAll", mybir.AluOpType.bypass,
                replica_groups=[list(range(NCORES))],
                ins=[ybuf.rearrange("a b c d e -> a (b c d e)").opt()],
                outs=[arecv.rearrange("a b c d e f -> a (b c d e f)").opt()],
            )
            if dbg:
                nc.sync.dma_start(out=ydbg[:], in_=ybuf[:])

            # ---------------- phase M: mode mix ----------------
            # yb rows i = (d*2+ri)*32 + h*16 + c_local  (blade-major);
            # y'_g = pg[g]^T yb restores true input scale and permutes
            # blades so that pod_g[ol] = sum_k tall[k, pos, ol] y'_g[k].
            with (
                tc.tile_pool(name="msb", bufs=3) as msb,
                tc.tile_pool(name="mps", bufs=2, space="PSUM") as mps,
                tc.tile_pool(name="mpy", bufs=1, space="PSUM") as mpy,
            ):
                tallv = tall.rearrange("i (p o) -> i p o", o=32)
                for half in range(4):  # 2 rows -> 128 positions each
                    # matmul psum outputs must sit at base partition 0/32/64,
                    # so the 128 o-blades live in two 64-partition tiles
                    podA = mps.tile([64, 512], FP32, tag="podA")
                    podB = mps.tile([64, 512], FP32, tag="podB")
                    pods = [podA, podB]
                    for rr in range(2):
                        r = half * 2 + rr
                        yb = msb.tile([128, 256], FP32, tag="yb")
                        for b in range(4):
                            for h in range(2):
                                for d in range(2):
                                    for ri in range(2):
                                        p0 = (d * 2 + ri) * 32 + h * CH
                                        nc.sync.dma_start(
                                            out=yb[p0:p0 + CH,
                                                   b * 64:(b + 1) * 64],
                                            in_=arecv[h * 4 + b, r, d, ri])
                        if dbg:
                            nc.sync.dma_start(out=yhdbg[r], in_=yb[:])
                        ygv = []
                        for g in range(4):
                            pyg = mpy.tile([128, 256], FP32, tag=f"pyg{g}")
                            nc.tensor.matmul(pyg[:], lhsT=pgs[g][:], rhs=yb[:],
                                             start=True, stop=True)
                            ygs = msb.tile([128, 256], MIX_DT, tag=f"ygs{g}")
                            nc.vector.tensor_copy(ygs[:], pyg[:])
                            ygv.append(ygs.rearrange("i (b m) -> i b m", b=4))
                        for m2 in range(M2):
                            p4 = (rr * 64 + m2) * 4
                            pos = r * M2 + m2
                            for g in range(4):
                                po = pods[g // 2]
                                q = 32 * (g % 2)
                                nc.tensor.matmul(
                                    po[q:q + 32, p4:p4 + 4],
                                    lhsT=tallv[:, pos, :],
                                    rhs=ygv[g][:, :, m2],
                                    start=True, stop=True)
                    sod = msb.tile([128, 512], FP32, tag="sod")
                    sodv = sod.rearrange("o (b p) -> o b p", b=4)
                    for gh in range(2):
                        nc.vector.tensor_copy(
                            sodv[64 * gh:64 * gh + 64],
                            pods[gh].rearrange("o (p b) -> o p b", p=128)
                                    .rearrange("o p b -> o b p"))
                    if dbg:
                        nc.sync.dma_start(out=sdbg[half], in_=sod[:])
                    for dst in range(8):
                        bp, ohp = dst % 4, dst // 4
                        for bt in range(4):
                            p0 = bt * 32 + ohp * OH
                            nc.sync.dma_start(
                                out=bsend[dst, bt, :, half, :],
                                in_=sod[p0:p0 + OH, bp * 128:(bp + 1) * 128])

            nc.gpsimd.collective_compute(
                "AllToAll", mybir.AluOpType.bypass,
                replica_groups=[list(range(NCORES))],
                ins=[bsend.rearrange("a b c d e -> a (b c d e)").opt()],
                outs=[brecv.rearrange("a b c d e -> a (b c d e)").opt()],
            )
            if dbg:
                nc.sync.dma_start(out=bdbg[:], in_=brecv[:])

            # ---------------- phase I: inverse DFT ----------------
            with (
                tc.tile_pool(name="isb", bufs=3) as isb,
                tc.tile_pool(name="ips", bufs=2, space="PSUM") as ips,
                tc.tile_pool(name="ops", bufs=1, space="PSUM") as ops,
            ):
                for ol in range(OH):
                    pos = []  # psum_o[d][hb]
                    for d in range(2):
                        ods = isb.tile([128, 64], FP32, tag="ods")
                        for u in range(2):
                            for sc in range(8):
                                nc.sync.dma_start(
                                    out=ods[u * 64 + sc * 8:u * 64 + sc * 8 + 8, :],
                                    in_=brecv[sc, 2 * d + u, ol])
                        row = []
                        for hb in range(2):
                            pv = ips.tile([128, 128], FP32, tag="pv")
                            nc.tensor.matmul(pv[:, 0:64], lhsT=ias[hb, 0][:],
                                             rhs=ods[:], start=True, stop=True)
                            nc.tensor.matmul(pv[:, 64:128], lhsT=ias[hb, 1][:],
                                             rhs=ods[:], start=True, stop=True)
                            sv = isb.tile([128, 128], FP32, tag="sv")
                            nc.vector.tensor_copy(sv[:], pv[:])
                            pvt = ips.tile([128, 128], FP32, tag="pvt")
                            nc.tensor.transpose(pvt[:], sv[:], ids[:])
                            svt = isb.tile([128, 128], FP32, tag="svt")
                            nc.vector.tensor_copy(svt[:], pvt[:])
                            po = ops.tile([128, 512], FP32, tag=f"po{d}{hb}")
                            nc.tensor.matmul(po[:, 0:256], lhsT=svt[:],
                                             rhs=ibs[0][:], start=True, stop=True)
                            nc.tensor.matmul(po[:, 256:512], lhsT=svt[:],
                                             rhs=ibs[1][:], start=True, stop=True)
                            row.append(po)
                        pos.append(row)
                    for hb in range(2):
                        so = isb.tile([128, W * 4], FP32, tag="so")
                        sov = so.rearrange("p (w k) -> p w k", k=4)
                        nc.vector.tensor_copy(sov[:, :, 0], pos[0][hb][:, 0:256])
                        nc.vector.tensor_copy(sov[:, :, 3], pos[0][hb][:, 256:512])
                        nc.vector.tensor_copy(sov[:, :, 1], pos[1][hb][:, 0:256])
                        nc.vector.tensor_copy(sov[:, :, 2], pos[1][hb][:, 256:512])
                        # per-row absmax int8 quantization
                        mt = isb.tile([128, 1], FP32, tag="mt")
                        nc.vector.tensor_reduce(
                            out=mt[:], in_=so[:], axis=mybir.AxisListType.X,
                            op=mybir.AluOpType.max, apply_absolute_value=True)
                        mtc = isb.tile([128, 1], FP32, tag="mtc")
                        nc.vector.tensor_scalar_max(mtc[:], mt[:], 1e-30)
                        rt = isb.tile([128, 1], FP32, tag="rt")
                        nc.vector.reciprocal(rt[:], mtc[:])
                        oqt = isb.tile([128, W * 4], INT8, tag="oqt")
                        nc.vector.tensor_scalar(
                            out=oqt[:], in0=so[:], scalar1=rt[:],
                            scalar2=127.0, op0=mybir.AluOpType.mult,
                            op1=mybir.AluOpType.mult)
                        nc.sync.dma_start(
                            out=oq[ol, hb * 128:(hb + 1) * 128].rearrange(
                                "h w k -> h (w k)"),
                            in_=oqt[:])
                        nc.sync.dma_start(out=osc[ol, hb], in_=mtc[:, 0])
    return nc


LAST_EXEC_NS = None
LAST_RUN_WALL_NS = None


def _build_runner(dbg):
    """Emit + compile the bass program once; return a cached jitted callable.

    Mirrors bass2jax.run_bass_via_pjrt's multi-core path, but the
    jax.jit(shard_map(...)) object is built a single time so warm calls
    skip retrace/relower/recompile, and no zero-filled output donations
    are shipped (the kernel writes every output byte).
    """
    from jax.experimental.shard_map import shard_map
    from jax.sharding import Mesh, PartitionSpec

    _b2j.install_neuronx_cc_hook()

    nc = bacc.Bacc("TRN2", target_bir_lowering=False, debug=False,
                   enable_asserts=False, num_devices=NCORES)
    _emit(nc, dbg=dbg)
    nc.compile()

    partition_name = (nc.partition_id_tensor.name
                      if nc.partition_id_tensor else None)
    in_names, out_names, out_avals = [], [], []
    for alloc in nc.m.functions[0].allocations:
        if not isinstance(alloc, mybir.MemoryLocationSet):
            continue
        name = alloc.memorylocations[0].name
        if alloc.kind == "ExternalInput":
            if name != partition_name:
                in_names.append(name)
        elif alloc.kind == "ExternalOutput":
            out_names.append(name)
            out_avals.append(jax.core.ShapedArray(
                tuple(alloc.tensor_shape), mybir.dt.np(alloc.dtype)))
    bind_names = list(in_names)
    if partition_name is not None:
        bind_names.append(partition_name)

    def _body(*args):
        operands = list(args)
        if partition_name is not None:
            operands.append(_b2j.partition_id_tensor())
        outs = _b2j._bass_exec_p.bind(
            *operands,
            out_avals=tuple(out_avals),
            in_names=tuple(bind_names),
            out_names=tuple(out_names),
            lowering_input_output_aliases=(),
            sim_require_finite=True,
            sim_require_nnan=True,
            nc=nc,
        )
        return tuple(outs)

    devices = jax.devices()[:NCORES]
    assert len(devices) == NCORES
    mesh = Mesh(np.asarray(devices), ("core",))
    sharded = jax.jit(
        shard_map(
            _body, mesh=mesh,
            in_specs=(PartitionSpec("core"),) * len(in_names),
            out_specs=(PartitionSpec("core"),) * len(out_names),
            check_rep=False,
        )
    )
    return sharded, in_names, out_names


def _get_runner(dbg=False):
    r = _prep_cache.get(dbg)
    if r is None:
        r = _build_runner(dbg)
        _prep_cache[dbg] = r
    return r


_wq_cache = {}


def _device_wq(wq_g, weights_key):
    """Weight-derived tensors are module parameters: keep them resident on
    device across calls (re-upload only when the weight bytes change)."""
    from jax.sharding import Mesh, PartitionSpec, NamedSharding
    ent = _wq_cache.get("wq")
    if ent is not None and ent[0] == weights_key:
        return ent[1]
    mesh = Mesh(np.asarray(jax.devices()[:NCORES]), ("core",))
    sh = NamedSharding(mesh, PartitionSpec("core"))
    dev = jax.device_put(wq_g, sh)
    dev.block_until_ready()
    _wq_cache["wq"] = (weights_key, dev)
    return dev


def kernel(x, weights, _dbg=False):
    global LAST_EXEC_NS, LAST_RUN_WALL_NS

    xs_g, wq_g, sv_g, s_w = _quantize_inputs(x, weights)
    sharded, in_names, out_names = _get_runner(_dbg)
    import hashlib
    wkey = hashlib.sha1(wq_g.tobytes()).hexdigest()
    ins = {"xs": xs_g, "wq": _device_wq(wq_g, wkey), "sv": sv_g}

    import time as _time
    res = None
    for attempt in range(3):
        try:
            _t0 = _time.perf_counter()
            out_arrs = sharded(*[ins[n] for n in in_names])
            res = {n: np.asarray(a) for n, a in zip(out_names, out_arrs)}
            LAST_RUN_WALL_NS = int((_time.perf_counter() - _t0) * 1e9)
            break
        except Exception:
            # transient device wedges (NRT_EXEC_UNIT_UNRECOVERABLE /
            # mesh desync) clear after a pause; retry before giving up
            if attempt == 2:
                raise
            _time.sleep(15)
    LAST_EXEC_NS = None

    oq_g = res["oq"].reshape(NCORES, OH, H, W, 4).astype(np.float32)
    osc_g = res["osc"].reshape(NCORES, OH, 2, 128) * (s_w / 127.0)
    out = np.empty((B, COUT, H, W, 4), np.float32)
    for k in range(NCORES):
        b, half = k % 4, k // 4
        scale = osc_g[k].reshape(OH, H, 1, 1)  # (OH, 2*128) rows == H
        out[b, half * OH:(half + 1) * OH] = oq_g[k] * scale
    if _dbg:
        return out, res
    return out


if __name__ == "__main__":
    xs = np.random.randn(B, CIN, H, W, 4).astype(np.float32)
    ws = np.random.rand(4, COUT, CIN, M2, M2).astype(np.float32) / (CIN * COUT)
    out = kernel(xs, ws)
    print(out.shape, out.dtype)


# revision 13
# speedup vs baseline: 1.0134x; 1.0134x over previous
"""CliffordSpectralConv2d on 8 trn2 NeuronCores (axon-tunneled).

Math: per sample b and "dual pair" (d1 = x0 + i*x3, d2 = x1 + i*x2):
    Y_d   = A @ X_d @ A^T          (crop-DFT, A = F256[rows 0:32 + 224:256])
    OD1   = sum_c W1*Y1 + W2*conj(Y2)   (positionwise over the 64x64 modes)
    OD2   = sum_c W1*Y2 + W2*conj(Y1)   (W1 = w0 + i*w3, W2 = w1 + i*w2)
    out_d = (1/65536) A^H @ OD_d @ conj(A)
with out components (re(o1), re(o2), im(o2), im(o1)).

Sharding (8 cores, one NEFF, SPMD):
  core k = (b = k%4, half = k//4)
  phase F: forward crop-DFT for x[b, 16*half:16*half+16] (32 complex ch)
  AllToAll #1 (1 MB/rank): reshard Y by mode-row slices
  phase M: per mode-row, y'_g = pg[g]^T y (4 signed-permutation matmuls,
           with per-channel input-dequant scales folded into pg), then
           positionwise contraction against the compact int8 weight table
           Tall[(s,c), (pos,o)] -- built host-side with NO 4x blade
           expansion (16.8 MB total over the wire instead of 134 MB).
  AllToAll #2 (1 MB/rank): reshard OD by (b, out-channel-half)
  phase I: inverse DFT for 16 output channels, interleave components,
           quantize each [128, W*4] row-tile to int8 with a per-row
           absmax scale (scales returned separately).

Wire-format (the axon tunnel runs at ~35 MB/s, so bytes == time):
  H2D per call: xs int8 (33.5 MB, per-(channel,pair) scales folded into
  pg), wq int8 (16.8 MB), pg fp32 (2 MB).  D2H: oq int8 (33.5 MB) +
  oscale fp32 (0.13 MB).  DFT matrices ride inside the NEFF as Const
  tensors; the jitted executable is cached across calls.
"""

import numpy as np

import jax

import concourse.mybir as mybir
import concourse.tile as tile
from concourse import bacc
from concourse import bass2jax as _b2j

NCORES = 8
B, CIN, COUT, H, W = 4, 32, 32, 256, 256
M = 32            # modes per corner
M2 = 64           # 2*M
CH = 16           # channels per core (forward)
OH = 16           # out channels per core (inverse)
ROWS = 8          # mode rows per core (mix)
POS = ROWS * M2   # positions per core (512)

FP32 = mybir.dt.float32
BF16 = mybir.dt.bfloat16
INT8 = mybir.dt.int8

MIX_DT = BF16     # mode-mix matmul dtype

# baseline grid: (matrix index in [W1r, W1i, W2r, W2i] order, sign),
# rows = input blade b in [Y1r, Y1i, Y2r, Y2i], cols = output blade g in
# [OD1r, OD1i, OD2r, OD2i].  W1r=w[0], W1i=w[3], W2r=w[1], W2i=w[2].
GRID = [
    [(0, 1.0), (1, 1.0), (2, 1.0), (3, 1.0)],
    [(1, -1.0), (0, 1.0), (3, 1.0), (2, -1.0)],
    [(2, 1.0), (3, 1.0), (0, 1.0), (1, 1.0)],
    [(3, 1.0), (2, -1.0), (1, -1.0), (0, 1.0)],
]

_prep_cache = {}


def _dft_mats():
    k = np.arange(H)
    sel = np.concatenate([np.arange(M), np.arange(H - M, H)])
    F = np.exp(-2j * np.pi * np.outer(k, k) / H)
    A = F[sel, :]
    return A.real.astype(np.float32).copy(), A.imag.astype(np.float32).copy()


def _host_consts():
    Ar, Ai = _dft_mats()  # (64, 256)
    # rx[ck, 0] = [Ar_chunk^T | Ai_chunk^T]; rx[ck, 1] = [-Ai_chunk^T | Ar_chunk^T]
    rx = np.zeros((2, 2, 128, 128), np.float32)
    for ck in range(2):
        ArT = Ar[:, ck * 128:(ck + 1) * 128].T  # (128, 64)
        AiT = Ai[:, ck * 128:(ck + 1) * 128].T
        rx[ck, 0, :, :64], rx[ck, 0, :, 64:] = ArT, AiT
        rx[ck, 1, :, :64], rx[ck, 1, :, 64:] = -AiT, ArT
    # ia[hb, 0] = [Ar_chunk; Ai_chunk] rows; ia[hb, 1] = [-Ai_chunk; Ar_chunk]
    ia = np.zeros((2, 2, 128, 128), np.float32)
    for hb in range(2):
        Arc = Ar[:, hb * 128:(hb + 1) * 128]  # (64, 128)
        Aic = Ai[:, hb * 128:(hb + 1) * 128]
        ia[hb, 0, :64], ia[hb, 0, 64:] = Arc, Aic
        ia[hb, 1, :64], ia[hb, 1, 64:] = -Aic, Arc
    # ib[0] = [Ar; Ai]/65536 ; ib[1] = [-Ai; Ar]/65536   (128, 256)
    s = 1.0 / float(H * W)
    ib = np.zeros((2, 128, 256), np.float32)
    ib[0, :64], ib[0, 64:] = Ar * s, Ai * s
    ib[1, :64], ib[1, 64:] = -Ai * s, Ar * s
    ident = np.eye(128, dtype=np.float32)
    return rx, ia, ib, ident


def _quantize_inputs(x, weights):
    """int8 wire format + the dequant data folded where the device needs it.

    Returns (xs_global, wq_global, pg_global, s_w) where the *_global arrays
    are the concatenated per-core shards for shard_map axis 0.
    """
    x = np.asarray(x, np.float32)
    w = np.asarray(weights, np.float32)

    # per-(channel, dual-pair) scales; pair d1 = comps (0,3), d2 = (1,2)
    ax = np.abs(x)
    m03 = np.maximum(ax[..., 0], ax[..., 3]).max(axis=(0, 2, 3))  # (C,)
    m12 = np.maximum(ax[..., 1], ax[..., 2]).max(axis=(0, 2, 3))
    s_cd = np.stack([m03, m12], axis=1) / 127.0                   # (C, 2)
    s_cd = np.maximum(s_cd, 1e-30)
    sc = s_cd[:, [0, 1, 1, 0]]                                    # per comp k
    xq = np.clip(np.round(x / sc[None, :, None, None, :]), -127, 127)
    xq = xq.astype(np.int8)
    # core k = (b = k%4, half = k//4) -> rows [k*CH:(k+1)*CH]
    xs_global = np.empty((NCORES * CH, H, W, 4), np.int8)
    for k in range(NCORES):
        b, half = k % 4, k // 4
        xs_global[k * CH:(k + 1) * CH] = xq[b, half * CH:(half + 1) * CH]

    # compact weight table: tall[s*32+c, pos*32+o] = w_sel[s][o, c, m1, m2]
    w_sel = w[[0, 3, 1, 2]]                                       # (4, O, C, 64, 64)
    s_w = max(float(np.abs(w_sel).max()) / 127.0, 1e-30)
    wq = np.clip(np.round(w_sel / s_w), -127, 127).astype(np.int8)
    t = np.ascontiguousarray(wq.transpose(0, 2, 3, 4, 1))         # (s, c, m1, m2, o)
    t = t.reshape(128, 64 * M2, 32)                               # rows (s,c), (pos, o)
    wq_global = np.empty((NCORES * 128, POS * 32), np.int8)
    for k in range(NCORES):
        wq_global[k * 128:(k + 1) * 128] = (
            t[:, POS * k:POS * (k + 1), :].reshape(128, POS * 32))

    # per-row dequant scales for the device-side signed-permutation const:
    # yb row i = (blade b)*32 + c carries scale s_cd[c, b//2]
    svec = np.empty((128, 1), np.float32)
    for i in range(128):
        svec[i, 0] = s_cd[i % 32, (i // 32) // 2]
    sv_global = np.tile(svec, (NCORES, 1, 1)).reshape(NCORES * 128, 1)

    return xs_global, wq_global, np.ascontiguousarray(sv_global), s_w


def _pg_pattern():
    """pg[g][i, k]: +-1 blade permutation, i = b*32+c, k = s*32+c."""
    pg = np.zeros((4, 128, 128), np.float32)
    for bb in range(4):
        for g in range(4):
            si, sign = GRID[bb][g]
            for c in range(32):
                pg[g, bb * 32 + c, si * 32 + c] = sign
    return pg


def _emit(nc, dbg=False):
    """Emit the SPMD program (same for every core; data differs)."""
    xs = nc.dram_tensor("xs", [CH, H, W, 4], INT8, kind="ExternalInput").ap()
    wq = nc.dram_tensor("wq", [128, POS * 32], INT8, kind="ExternalInput").ap()
    sv = nc.dram_tensor("sv", [128, 1], FP32, kind="ExternalInput").ap()
    pgt = nc.inline_tensor(_pg_pattern(), name="pgc").ap()
    oq = nc.dram_tensor("oq", [OH, H, W, 4], INT8, kind="ExternalOutput").ap()
    osc = nc.dram_tensor("osc", [OH, 2, 128], FP32, kind="ExternalOutput").ap()

    rxc, iac, ibc, identc = _host_consts()
    rx = nc.inline_tensor(rxc, name="rxc").ap()
    ia = nc.inline_tensor(iac, name="iac").ap()
    ib = nc.inline_tensor(ibc, name="ibc").ap()
    ident = nc.inline_tensor(identc, name="identc").ap()

    if dbg:
        ydbg = nc.dram_tensor("ydbg", [64, 2, 2, CH, M2], FP32, kind="ExternalOutput").ap()
        yhdbg = nc.dram_tensor("yhdbg", [ROWS, 128, 256], FP32, kind="ExternalOutput").ap()
        sdbg = nc.dram_tensor("sdbg", [4, 128, 512], FP32, kind="ExternalOutput").ap()
        bdbg = nc.dram_tensor("bdbg", [8, 4, OH, 8, M2], FP32, kind="ExternalOutput").ap()

    with tile.TileContext(nc) as tc:
        with (
            tc.tile_pool(name="consts", bufs=1) as cpool,
            tc.tile_pool(name="dram", bufs=1, space="DRAM") as dpool,
        ):
            # resident constants (partition dim must be first -> one tile each)
            rxs, ias = {}, {}
            for ck in range(2):
                for j in range(2):
                    t = cpool.tile([128, 128], FP32, name=f"rxs{ck}{j}")
                    nc.sync.dma_start(out=t[:], in_=rx[ck, j])
                    rxs[ck, j] = t
                    t2 = cpool.tile([128, 128], FP32, name=f"ias{ck}{j}")
                    nc.sync.dma_start(out=t2[:], in_=ia[ck, j])
                    ias[ck, j] = t2
            ibs = {}
            for j in range(2):
                t = cpool.tile([128, 256], FP32, name=f"ibs{j}")
                nc.sync.dma_start(out=t[:], in_=ib[j])
                ibs[j] = t
            ids = cpool.tile([128, 128], FP32, name="ids")
            nc.sync.dma_start(out=ids[:], in_=ident[:])
            svt = cpool.tile([128, 1], FP32, name="svt")
            nc.sync.dma_start(out=svt[:], in_=sv[:])
            pgs = {}
            for g in range(4):
                tb = cpool.tile([128, 128], FP32, name=f"pgb{g}")
                nc.sync.dma_start(out=tb[:], in_=pgt[g])
                t = cpool.tile([128, 128], FP32, name=f"pgs{g}")
                nc.vector.tensor_scalar(
                    out=t[:], in0=tb[:], scalar1=svt[:], scalar2=None,
                    op0=mybir.AluOpType.mult)
                pgs[g] = t
            # compact weight table -> resident bf16 Tall
            tall = cpool.tile([128, POS * 32], MIX_DT, name="tall")
            with tc.tile_pool(name="wload", bufs=1) as wl:
                ti = wl.tile([128, POS * 32], INT8, name="ti")
                nc.sync.dma_start(out=ti[:], in_=wq[:])
                nc.vector.tensor_copy(tall[:], ti[:])

            # collective buffers
            ybuf = dpool.tile([64, 2, 2, CH, M2], FP32, name="ybuf")
            arecv = dpool.tile([8, ROWS, 2, 2, CH, M2], FP32, name="arecv")
            bsend = dpool.tile([8, 4, OH, 4, 128], FP32, name="bsend")
            brecv = dpool.tile([8, 4, OH, 8, M2], FP32, name="brecv")

            # ---------------- phase F: forward crop-DFT ----------------
            with (
                tc.tile_pool(name="fsb", bufs=3) as fsb,
                tc.tile_pool(name="ftt", bufs=2) as ftt,
                tc.tile_pool(name="fps", bufs=2, space="PSUM") as fps,
            ):
                for c in range(CH):
                    xt0i = fsb.tile([128, W * 4], INT8, tag="xt0i")
                    nc.sync.dma_start(
                        out=xt0i[:], in_=xs[c, 0:128].rearrange("h w k -> h (w k)"))
                    xt0 = fsb.tile([128, W * 4], FP32, tag="xt0")
                    nc.vector.tensor_copy(xt0[:], xt0i[:])
                    xt1i = fsb.tile([128, W * 4], INT8, tag="xt1i")
                    nc.sync.dma_start(
                        out=xt1i[:], in_=xs[c, 128:256].rearrange("h w k -> h (w k)"))
                    xt1 = fsb.tile([128, W * 4], FP32, tag="xt1")
                    nc.vector.tensor_copy(xt1[:], xt1i[:])
                    xv = [xt0.rearrange("h (w k) -> h k w", k=4),
                          xt1.rearrange("h (w k) -> h k w", k=4)]
                    for d in range(2):
                        re_c, im_c = (0, 3) if d == 0 else (1, 2)
                        tts = []
                        for wb in range(2):
                            pt = fps.tile([128, 128], FP32, tag="pt")
                            for hk in range(2):
                                nc.tensor.matmul(
                                    pt[:],
                                    lhsT=xv[hk][:, re_c, wb * 128:(wb + 1) * 128],
                                    rhs=rxs[hk, 0][:],
                                    start=(hk == 0), stop=False)
                                nc.tensor.matmul(
                                    pt[:],
                                    lhsT=xv[hk][:, im_c, wb * 128:(wb + 1) * 128],
                                    rhs=rxs[hk, 1][:],
                                    start=False, stop=(hk == 1))
                            tt = ftt.tile([128, 128], FP32, tag=f"tt{wb}")
                            nc.vector.tensor_copy(tt[:], pt[:])
                            tts.append(tt)
                        py = fps.tile([64, 128], FP32, tag="py")
                        for wb in range(2):
                            nc.tensor.matmul(
                                py[:], lhsT=tts[wb][:, 0:64], rhs=rxs[wb, 0][:],
                                start=(wb == 0), stop=False)
                            nc.tensor.matmul(
                                py[:], lhsT=tts[wb][:, 64:128], rhs=rxs[wb, 1][:],
                                start=False, stop=(wb == 1))
                        sy = ftt.tile([64, 128], FP32, tag="sy")
                        nc.vector.tensor_copy(sy[:], py[:])
                        nc.sync.dma_start(
                            out=ybuf[:, d, :, c, :],
                            in_=sy.rearrange("m (r n) -> m r n", r=2))

            nc.gpsimd.collective_compute(
                "AllToAll", mybir.AluOpType.bypass,
                replica_groups=[list(range(NCORES))],
                ins=[ybuf.rearrange("a b c d e -> a (b c d e)").opt()],
                outs=[arecv.rearrange("a b c d e f -> a (b c d e f)").opt()],
            )
            if dbg:
                nc.sync.dma_start(out=ydbg[:], in_=ybuf[:])

            # ---------------- phase M: mode mix ----------------
            # yb rows i = (d*2+ri)*32 + h*16 + c_local  (blade-major);
            # y'_g = pg[g]^T yb restores true input scale and permutes
            # blades so that pod_g[ol] = sum_k tall[k, pos, ol] y'_g[k].
            with (
                tc.tile_pool(name="msb", bufs=3) as msb,
                tc.tile_pool(name="mps", bufs=2, space="PSUM") as mps,
                tc.tile_pool(name="mpy", bufs=1, space="PSUM") as mpy,
            ):
                tallv = tall.rearrange("i (p o) -> i p o", o=32)
                for half in range(4):  # 2 rows -> 128 positions each
                    # matmul psum outputs must sit at base partition 0/32/64,
                    # so the 128 o-blades live in two 64-partition tiles
                    podA = mps.tile([64, 512], FP32, tag="podA")
                    podB = mps.tile([64, 512], FP32, tag="podB")
                    pods = [podA, podB]
                    for rr in range(2):
                        r = half * 2 + rr
                        yb = msb.tile([128, 256], FP32, tag="yb")
                        for b in range(4):
                            for h in range(2):
                                for d in range(2):
                                    for ri in range(2):
                                        p0 = (d * 2 + ri) * 32 + h * CH
                                        nc.sync.dma_start(
                                            out=yb[p0:p0 + CH,
                                                   b * 64:(b + 1) * 64],
                                            in_=arecv[h * 4 + b, r, d, ri])
                        if dbg:
                            nc.sync.dma_start(out=yhdbg[r], in_=yb[:])
                        ygv = []
                        for g in range(4):
                            pyg = mpy.tile([128, 256], FP32, tag=f"pyg{g}")
                            nc.tensor.matmul(pyg[:], lhsT=pgs[g][:], rhs=yb[:],
                                             start=True, stop=True)
                            ygs = msb.tile([128, 256], MIX_DT, tag=f"ygs{g}")
                            nc.vector.tensor_copy(ygs[:], pyg[:])
                            ygv.append(ygs.rearrange("i (b m) -> i b m", b=4))
                        for m2 in range(M2):
                            p4 = (rr * 64 + m2) * 4
                            pos = r * M2 + m2
                            for g in range(4):
                                po = pods[g // 2]
                                q = 32 * (g % 2)
                                nc.tensor.matmul(
                                    po[q:q + 32, p4:p4 + 4],
                                    lhsT=tallv[:, pos, :],
                                    rhs=ygv[g][:, :, m2],
                                    start=True, stop=True)
                    sod = msb.tile([128, 512], FP32, tag="sod")
                    sodv = sod.rearrange("o (b p) -> o b p", b=4)
                    for gh in range(2):
                        nc.vector.tensor_copy(
                            sodv[64 * gh:64 * gh + 64],
                            pods[gh].rearrange("o (p b) -> o p b", p=128)
                                    .rearrange("o p b -> o b p"))
                    if dbg:
                        nc.sync.dma_start(out=sdbg[half], in_=sod[:])
                    for dst in range(8):
                        bp, ohp = dst % 4, dst // 4
                        for bt in range(4):
                            p0 = bt * 32 + ohp * OH
                            nc.sync.dma_start(
                                out=bsend[dst, bt, :, half, :],
                                in_=sod[p0:p0 + OH, bp * 128:(bp + 1) * 128])

            nc.gpsimd.collective_compute(
                "AllToAll", mybir.AluOpType.bypass,
                replica_groups=[list(range(NCORES))],
                ins=[bsend.rearrange("a b c d e -> a (b c d e)").opt()],
                outs=[brecv.rearrange("a b c d e -> a (b c d e)").opt()],
            )
            if dbg:
                nc.sync.dma_start(out=bdbg[:], in_=brecv[:])

            # ---------------- phase I: inverse DFT ----------------
            with (
                tc.tile_pool(name="isb", bufs=3) as isb,
                tc.tile_pool(name="ips", bufs=2, space="PSUM") as ips,
                tc.tile_pool(name="ops", bufs=1, space="PSUM") as ops,
            ):
                for ol in range(OH):
                    pos = []  # psum_o[d][hb]
                    for d in range(2):
                        ods = isb.tile([128, 64], FP32, tag="ods")
                        for u in range(2):
                            for sc in range(8):
                                nc.sync.dma_start(
                                    out=ods[u * 64 + sc * 8:u * 64 + sc * 8 + 8, :],
                                    in_=brecv[sc, 2 * d + u, ol])
                        row = []
                        for hb in range(2):
                            pv = ips.tile([128, 128], FP32, tag="pv")
                            nc.tensor.matmul(pv[:, 0:64], lhsT=ias[hb, 0][:],
                                             rhs=ods[:], start=True, stop=True)
                            nc.tensor.matmul(pv[:, 64:128], lhsT=ias[hb, 1][:],
                                             rhs=ods[:], start=True, stop=True)
                            sv = isb.tile([128, 128], FP32, tag="sv")
                            nc.vector.tensor_copy(sv[:], pv[:])
                            pvt = ips.tile([128, 128], FP32, tag="pvt")
                            nc.tensor.transpose(pvt[:], sv[:], ids[:])
                            svt = isb.tile([128, 128], FP32, tag="svt")
                            nc.vector.tensor_copy(svt[:], pvt[:])
                            po = ops.tile([128, 512], FP32, tag=f"po{d}{hb}")
                            nc.tensor.matmul(po[:, 0:256], lhsT=svt[:],
                                             rhs=ibs[0][:], start=True, stop=True)
                            nc.tensor.matmul(po[:, 256:512], lhsT=svt[:],
                                             rhs=ibs[1][:], start=True, stop=True)
                            row.append(po)
                        pos.append(row)
                    for hb in range(2):
                        so = isb.tile([128, W * 4], FP32, tag="so")
                        sov = so.rearrange("p (w k) -> p w k", k=4)
                        nc.vector.tensor_copy(sov[:, :, 0], pos[0][hb][:, 0:256])
                        nc.vector.tensor_copy(sov[:, :, 3], pos[0][hb][:, 256:512])
                        nc.vector.tensor_copy(sov[:, :, 1], pos[1][hb][:, 0:256])
                        nc.vector.tensor_copy(sov[:, :, 2], pos[1][hb][:, 256:512])
                        # per-row absmax int8 quantization
                        mt = isb.tile([128, 1], FP32, tag="mt")
                        nc.vector.tensor_reduce(
                            out=mt[:], in_=so[:], axis=mybir.AxisListType.X,
                            op=mybir.AluOpType.max, apply_absolute_value=True)
                        mtc = isb.tile([128, 1], FP32, tag="mtc")
                        nc.vector.tensor_scalar_max(mtc[:], mt[:], 1e-30)
                        rt = isb.tile([128, 1], FP32, tag="rt")
                        nc.vector.reciprocal(rt[:], mtc[:])
                        oqt = isb.tile([128, W * 4], INT8, tag="oqt")
                        nc.vector.tensor_scalar(
                            out=oqt[:], in0=so[:], scalar1=rt[:],
                            scalar2=127.0, op0=mybir.AluOpType.mult,
                            op1=mybir.AluOpType.mult)
                        nc.sync.dma_start(
                            out=oq[ol, hb * 128:(hb + 1) * 128].rearrange(
                                "h w k -> h (w k)"),
                            in_=oqt[:])
                        nc.sync.dma_start(out=osc[ol, hb], in_=mtc[:, 0])
    return nc


LAST_EXEC_NS = None
LAST_RUN_WALL_NS = None


def _build_runner(dbg):
    """Emit + compile the bass program once; return a cached jitted callable.

    Mirrors bass2jax.run_bass_via_pjrt's multi-core path, but the
    jax.jit(shard_map(...)) object is built a single time so warm calls
    skip retrace/relower/recompile, and no zero-filled output donations
    are shipped (the kernel writes every output byte).
    """
    from jax.experimental.shard_map import shard_map
    from jax.sharding import Mesh, PartitionSpec

    _b2j.install_neuronx_cc_hook()

    nc = bacc.Bacc("TRN2", target_bir_lowering=False, debug=False,
                   enable_asserts=False, num_devices=NCORES)
    _emit(nc, dbg=dbg)
    nc.compile()

    partition_name = (nc.partition_id_tensor.name
                      if nc.partition_id_tensor else None)
    in_names, out_names, out_avals = [], [], []
    for alloc in nc.m.functions[0].allocations:
        if not isinstance(alloc, mybir.MemoryLocationSet):
            continue
        name = alloc.memorylocations[0].name
        if alloc.kind == "ExternalInput":
            if name != partition_name:
                in_names.append(name)
        elif alloc.kind == "ExternalOutput":
            out_names.append(name)
            out_avals.append(jax.core.ShapedArray(
                tuple(alloc.tensor_shape), mybir.dt.np(alloc.dtype)))
    bind_names = list(in_names)
    if partition_name is not None:
        bind_names.append(partition_name)

    def _body(*args):
        operands = list(args)
        if partition_name is not None:
            operands.append(_b2j.partition_id_tensor())
        outs = _b2j._bass_exec_p.bind(
            *operands,
            out_avals=tuple(out_avals),
            in_names=tuple(bind_names),
            out_names=tuple(out_names),
            lowering_input_output_aliases=(),
            sim_require_finite=True,
            sim_require_nnan=True,
            nc=nc,
        )
        return tuple(outs)

    devices = jax.devices()[:NCORES]
    assert len(devices) == NCORES
    mesh = Mesh(np.asarray(devices), ("core",))
    sharded = jax.jit(
        shard_map(
            _body, mesh=mesh,
            in_specs=(PartitionSpec("core"),) * len(in_names),
            out_specs=(PartitionSpec("core"),) * len(out_names),
            check_rep=False,
        )
    )
    return sharded, in_names, out_names


def _get_runner(dbg=False):
    r = _prep_cache.get(dbg)
    if r is None:
        r = _build_runner(dbg)
        _prep_cache[dbg] = r
    return r


_wq_cache = {}


def _device_wq(wq_g, weights_key):
    """Weight-derived tensors are module parameters: keep them resident on
    device across calls (re-upload only when the weight bytes change)."""
    from jax.sharding import Mesh, PartitionSpec, NamedSharding
    ent = _wq_cache.get("wq")
    if ent is not None and ent[0] == weights_key:
        return ent[1]
    mesh = Mesh(np.asarray(jax.devices()[:NCORES]), ("core",))
    sh = NamedSharding(mesh, PartitionSpec("core"))
    dev = jax.device_put(wq_g, sh)
    dev.block_until_ready()
    _wq_cache["wq"] = (weights_key, dev)
    return dev


def kernel(x, weights, _dbg=False):
    global LAST_EXEC_NS, LAST_RUN_WALL_NS

    xs_g, wq_g, sv_g, s_w = _quantize_inputs(x, weights)
    sharded, in_names, out_names = _get_runner(_dbg)
    import hashlib
    wkey = hashlib.sha1(wq_g.tobytes()).hexdigest()
    ins = {"xs": xs_g, "wq": _device_wq(wq_g, wkey), "sv": sv_g}

    import time as _time
    res = None
    for attempt in range(3):
        try:
            _t0 = _time.perf_counter()
            out_arrs = sharded(*[ins[n] for n in in_names])
            res = {n: np.asarray(a) for n, a in zip(out_names, out_arrs)}
            LAST_RUN_WALL_NS = int((_time.perf_counter() - _t0) * 1e9)
            break
        except Exception:
            # transient device wedges (NRT_EXEC_UNIT_UNRECOVERABLE /
            # mesh desync) clear after a pause; retry before giving up
            if attempt == 2:
                raise
            _time.sleep(15)
    LAST_EXEC_NS = None

    oq_g = res["oq"].reshape(NCORES, OH, H, W, 4).astype(np.float32)
    osc_g = res["osc"].reshape(NCORES, OH, 2, 128) * (s_w / 127.0)
    out = np.empty((B, COUT, H, W, 4), np.float32)
    for k in range(NCORES):
        b, half = k % 4, k // 4
        scale = osc_g[k].reshape(OH, H, 1, 1)  # (OH, 2*128) rows == H
        out[b, half * OH:(half + 1) * OH] = oq_g[k] * scale
    if _dbg:
        return out, res
    return out


if __name__ == "__main__":
    xs = np.random.randn(B, CIN, H, W, 4).astype(np.float32)
    ws = np.random.rand(4, COUT, CIN, M2, M2).astype(np.float32) / (CIN * COUT)
    out = kernel(xs, ws)
    print(out.shape, out.dtype)
